# revision 1
# baseline (speedup 1.0000x reference)
"""ColorHistogramLoss (soft histogram EMD) on 8 Trainium2 NeuronCores.

Strategy: pure data parallel over batch (B=8 -> one batch element per core).
Each core computes, for its 3 channels x {pred, target}, the 64-bin soft
(Gaussian-weighted) histogram of its 384x384 image:

    hist[j] = sum_px exp(-(x_px - c_j)^2 / denom)

The Gaussian is evaluated on the Scalar (ACT) engine via
Derivative_Erf(scale*x + bias) = 2/sqrt(pi)*exp(-u^2) with the fused
accum_out free-dim reduction.  ACT instruction overhead is large
(~0.3-0.6us), so instead of one instruction per (bin, image-pair) at
FD=2304 (192 instructions), each channel image is laid out [16, 9216] and
replicated 8x across partition blocks (host-side tile), and a
PER-PARTITION bias AP makes each 16-row block evaluate a different bin:
one instruction covers 8 bins x one image at FD=9216.  48 instructions
total (6 images x 8 bin-octets), ~10% faster end-to-end than the
192-instruction layout (measured 366-369us vs 402-405us per iteration,
same process).  An on-device step-0 replicating DMA was tried and is
faster to feed, but intermittently corrupts results and can wedge the
device (NRT_EXEC_UNIT_UNRECOVERABLE) — do not reintroduce it.

Per-partition partial sums land in Hacc[128, 48]; one PE matmul against an
8-column block selector reduces over the 16 rows of each block, giving all
384 histogram values as [48, 8] in PSUM.  The tiny tail (normalize,
cumsum, |diff|, mean over 8*3*64) runs on host in float64.
"""

import functools
import math

import numpy as np

N_CORES = 8
NUM_BINS = 64
B, C, H, W = 8, 3, 384, 384
HW = H * W
N_UNITS = 2 * C                 # (channel, pred/target) images per core
N_OCT = NUM_BINS // 8           # 8 bin-octets; one ACT instruction each
FREE = HW // 16                 # channel image as [16, 9216]
DENOM = 2.0 * (1.0 / 64.0) ** 2 + 1e-7
SCALE = 1.0 / math.sqrt(DENOM)
DERF_SCALE = math.sqrt(math.pi) / 2.0  # Derivative_Erf = 2/sqrt(pi) * exp(-u^2)


def _build_program():
    import concourse.bass as bass
    import concourse.mybir as mybir

    nc = bass.Bass()
    xs = [
        nc.dram_tensor(f"x{u}", [128, FREE], mybir.dt.float32, kind="ExternalInput")
        for u in range(N_UNITS)
    ]
    cst = nc.dram_tensor("consts", [128, 16], mybir.dt.float32, kind="ExternalInput")
    hist_out = nc.dram_tensor(
        "hist", [N_UNITS * N_OCT, 8], mybir.dt.float32, kind="ExternalOutput"
    )

    with (
        nc.sbuf_tensor("xt0", [128, FREE], mybir.dt.float32) as xt0,
        nc.sbuf_tensor("xt1", [128, FREE], mybir.dt.float32) as xt1,
        nc.sbuf_tensor("xt2", [128, FREE], mybir.dt.float32) as xt2,
        nc.sbuf_tensor("cstt", [128, 16], mybir.dt.float32) as cstt,
        nc.sbuf_tensor("wscr", [128, FREE], mybir.dt.float32) as wscr,
        nc.sbuf_tensor("hacc", [128, N_UNITS * N_OCT], mybir.dt.float32) as hacc,
        nc.sbuf_tensor("ho", [N_UNITS * N_OCT, 8], mybir.dt.float32) as ho,
        nc.psum_tensor("ph", [N_UNITS * N_OCT, 8], mybir.dt.float32) as ph,
        nc.semaphore("sem_c") as sem_c,
        nc.semaphore("sem_x0") as sem_x0,
        nc.semaphore("sem_x1") as sem_x1,
        nc.semaphore("sem_x2") as sem_x2,
        nc.semaphore("act_sem") as act_sem,
        nc.semaphore("pe_sem") as pe_sem,
        nc.semaphore("cp_sem") as cp_sem,
        nc.Block() as block,
    ):
        slots = [xt0, xt1, xt2]
        xsems = [sem_x0, sem_x1, sem_x2]

        @block.sync
        def _(sync):
            sync.dma_start(out=cstt[:], in_=cst[:]).then_inc(sem_c, 16)
            for u in range(N_UNITS):
                slot = u % 3
                if u >= 3:
                    # slot is free once unit u-3's 8 ACT instructions are done
                    sync.wait_ge(act_sem, N_OCT * (u - 2))
                sync.dma_start(out=slots[slot][:], in_=xs[u][:]).then_inc(
                    xsems[slot], 16
                )
            sync.wait_ge(cp_sem, 1)
            sync.dma_start(out=hist_out[:], in_=ho[:]).then_inc(sem_c, 16)

        @block.scalar
        def _(scalar):
            # dummy activation on scratch: pulls the ACT table load (~2.7us)
            # forward so it overlaps with the input DMAs
            scalar.activation(
                wscr[0:128, 0:1], wscr[0:128, 1:2],
                mybir.ActivationFunctionType.Derivative_Erf,
                bias=wscr[:, 2:3], scale=1.0,
            )
            scalar.wait_ge(sem_c, 16)
            for u in range(N_UNITS):
                slot = u % 3
                scalar.wait_ge(xsems[slot], 16 * (u // 3 + 1))
                for o in range(N_OCT):
                    # partition block k (rows 16k..16k+15) evaluates bin 8o+k
                    scalar.activation(
                        wscr[:],
                        slots[slot][:],
                        mybir.ActivationFunctionType.Derivative_Erf,
                        bias=cstt[:, o : o + 1],
                        scale=float(SCALE),
                        accum_out=hacc[:, N_OCT * u + o : N_OCT * u + o + 1],
                    ).then_inc(act_sem, 1)

        @block.tensor
        def _(tensor):
            tensor.wait_ge(act_sem, N_UNITS * N_OCT)
            # ph[col, k] = sum_p hacc[p, col] * sel[p, k]  (sel: p//16 == k)
            tensor.matmul(
                ph[0 : N_UNITS * N_OCT, 0:8],
                hacc[:, :],
                cstt[:, 8:16],
                start=True,
                stop=True,
            ).then_inc(pe_sem, 1)

        @block.vector
        def _(vector):
            vector.wait_ge(pe_sem, 1)
            vector.tensor_copy(ho[:, :], ph[:, :]).then_inc(cp_sem, 1)

    return nc


def _make_consts():
    centers = np.linspace(0.0, 1.0, NUM_BINS, dtype=np.float32)
    bias = (-centers.astype(np.float64) * SCALE).astype(np.float32)
    cst = np.zeros((128, 16), dtype=np.float32)
    p = np.arange(128)
    for o in range(N_OCT):
        cst[:, o] = bias[8 * o + p // 16]      # per-partition bias: block k -> bin 8o+k
    for k in range(8):
        cst[p // 16 == k, 8 + k] = 1.0         # block selector for the PE reduce
    return cst


@functools.lru_cache(maxsize=1)
def _get_runner():
    """Compile the SPMD program once; return a callable list[in_map] -> list[out_map]."""
    import jax
    from jax.experimental.shard_map import shard_map
    from jax.sharding import Mesh, PartitionSpec

    from concourse import mybir
    from concourse.bass2jax import (
        _bass_exec_p,
        install_neuronx_cc_hook,
        partition_id_tensor,
    )

    nc = _build_program()
    install_neuronx_cc_hook()

    partition_name = (
        nc.partition_id_tensor.name if nc.partition_id_tensor else None
    )
    in_names, out_names, out_avals, zero_outs = [], [], [], []
    for alloc in nc.m.functions[0].allocations:
        if not isinstance(alloc, mybir.MemoryLocationSet):
            continue
        name = alloc.memorylocations[0].name
        if alloc.kind == "ExternalInput":
            if name != partition_name:
                in_names.append(name)
        elif alloc.kind == "ExternalOutput":
            out_names.append(name)
            shape = tuple(alloc.tensor_shape)
            dtype = mybir.dt.np(alloc.dtype)
            out_avals.append(jax.core.ShapedArray(shape, dtype))
            zero_outs.append(np.zeros(shape, dtype))
    n_params = len(in_names)
    n_outs = len(out_avals)
    all_in_names = list(in_names) + list(out_names)
    if partition_name is not None:
        all_in_names.append(partition_name)
    donate = tuple(range(n_params, n_params + n_outs))

    def _body(*args):
        operands = list(args)
        if partition_name is not None:
            operands.append(partition_id_tensor())
        outs = _bass_exec_p.bind(
            *operands,
            out_avals=tuple(out_avals),
            in_names=tuple(all_in_names),
            out_names=tuple(out_names),
            lowering_input_output_aliases=(),
            sim_require_finite=True,
            sim_require_nnan=True,
            nc=nc,
        )
        return tuple(outs)

    devices = jax.devices()[:N_CORES]
    mesh = Mesh(np.asarray(devices), ("core",))
    sharded = jax.jit(
        shard_map(
            _body,
            mesh=mesh,
            in_specs=(PartitionSpec("core"),) * (n_params + n_outs),
            out_specs=(PartitionSpec("core"),) * n_outs,
            check_rep=False,
        ),
        donate_argnums=donate,
        keep_unused=True,
    )

    class Runner:
        def __init__(self):
            self.sharded = sharded
            self.in_names = in_names
            self.out_names = out_names
            self.out_avals = out_avals
            self.zero_outs = zero_outs

        def concat_inputs(self, in_maps):
            return [
                np.concatenate([np.asarray(m[name]) for m in in_maps], axis=0)
                for name in in_names
            ]

        def fresh_zeros(self):
            return [
                np.zeros((N_CORES * z.shape[0], *z.shape[1:]), z.dtype)
                for z in zero_outs
            ]

        def split_outputs(self, out_arrs):
            return [
                {
                    name: np.asarray(out_arrs[i]).reshape(
                        N_CORES, *out_avals[i].shape
                    )[c]
                    for i, name in enumerate(out_names)
                }
                for c in range(N_CORES)
            ]

        def __call__(self, in_maps):
            out_arrs = self.sharded(*self.concat_inputs(in_maps), *self.fresh_zeros())
            return self.split_outputs(out_arrs)

    return Runner()


def _shard_inputs(pred, target):
    cst = _make_consts()
    maps = []
    for b in range(B):
        m = {"consts": cst}
        for c in range(C):
            for t, src in enumerate((pred, target)):
                u = 2 * c + t
                img = np.ascontiguousarray(src[b, c], dtype=np.float32).reshape(
                    16, FREE
                )
                m[f"x{u}"] = np.tile(img, (8, 1))
        maps.append(m)
    return maps


def _finish_on_host(results):
    total = 0.0
    for b in range(B):
        hist = results[b]["hist"].astype(np.float64) * DERF_SCALE
        for c in range(C):
            p = hist[N_OCT * (2 * c) : N_OCT * (2 * c) + N_OCT, :].reshape(NUM_BINS)
            t = hist[N_OCT * (2 * c + 1) : N_OCT * (2 * c + 1) + N_OCT, :].reshape(
                NUM_BINS
            )
            pn = p / (p.sum() + 1e-7)
            tn = t / (t.sum() + 1e-7)
            total += np.abs(np.cumsum(pn) - np.cumsum(tn)).sum()
    return np.float32(total / (B * C * NUM_BINS))


def kernel(pred, target):
    pred = np.asarray(pred, dtype=np.float32)
    target = np.asarray(target, dtype=np.float32)
    assert pred.shape == (B, C, H, W) and target.shape == (B, C, H, W)
    run = _get_runner()
    results = run(_shard_inputs(pred, target))
    return np.asarray(_finish_on_host(results), dtype=np.float32)



# revision 3
# speedup vs baseline: 1.3783x; 1.3783x over previous
"""ColorHistogramLoss (soft histogram EMD) on 8 Trainium2 NeuronCores.

Strategy: pure data parallel over batch (B=8 -> one batch element per core).
Each core computes, for its 3 channels x {pred, target}, the 64-bin soft
(Gaussian-weighted) histogram of its 384x384 image.

Dense work = 64 Gaussian evals per pixel.  The ACT (scalar) engine evaluates
Gaussians via Derivative_Erf(scale*x + bias) = 2/sqrt(pi)*exp(-u^2) with the
fused accum_out free-dim reduction; each image is laid out [16, FREE] and
replicated 8x across partition blocks so a PER-PARTITION bias makes one pass
cover 8 bins (block k of 16 partitions evaluates bin 8k+r for pass r).

NEW vs the pure-ACT baseline (342.7us): half of the 8 per-image passes are
offloaded to the otherwise-idle Vector engine (DVE) using the Gaussian ratio
recurrence
    w_{j+1}(x) = w_j(x) * tau(x) * C_j,   tau = exp((2s/D) x),
    C_j = exp(-s (c_j + c_{j+1})/D)
computed by ONE fused scalar_tensor_tensor pass per 8-bin round:
    out = (w_prev * C[per-partition]) * tau,  accum_out = sum(out).
ACT computes per image-quarter: tau (Exp), seeds r=0 and r=4 (Derivative_Erf),
and dense rounds r=3, r=7; DVE chains r=1,2 from the r=0 seed and r=5,6 from
the r=4 seed (chain length <= 2 keeps fp32 error ~1e-7 and short deps).
Exp and Derivative_Erf live in different ACT table sets (~2.7us per switch),
so passes are batched per image: [Exp: 4x tau][DErf: 4x(s0,s4,d3,d7)].

Work unit = image quarter [128, 2304] to fit all pipeline buffers in SBUF.
Per-partition partial sums land in hacc[128, 192]; two PE matmuls against a
block selector reduce over the 16 rows of each block; the tiny tail
(normalize, cumsum, |diff|, mean) runs on host in float64.
"""

import functools
import math

import numpy as np

N_CORES = 8
NUM_BINS = 64
B, C, H, W = 8, 3, 384, 384
HW = H * W
N_UNITS = 2 * C                 # (channel, pred/target) images per core
FREE = HW // 16                 # channel image as [16, 9216], replicated 8x
NQ = 4                          # quarters per image
QF = FREE // NQ                 # quarter free dim (2304)
UNITS = N_UNITS * NQ            # 24 work units per core per iteration

DENOM = 2.0 * (1.0 / 64.0) ** 2 + 1e-7
SCALE = 1.0 / math.sqrt(DENOM)          # Derivative_Erf arg scale
SPACING = 1.0 / 63.0                    # bin-center spacing
TAU_SCALE = 2.0 * SPACING / DENOM       # tau = exp(TAU_SCALE * x)
DERF_SCALE = math.sqrt(math.pi) / 2.0   # Derivative_Erf = 2/sqrt(pi)*exp(-u^2)

SEED_ROUNDS = (0, 4)            # ACT Derivative_Erf seeds
DENSE_ROUNDS = (3, 7)           # ACT Derivative_Erf dense rounds
DVE_ROUNDS = (1, 2, 5, 6)       # DVE recurrence rounds
ACT_PASSES_PER_IT = UNITS * 5   # tau + s0 + s4 + d3 + d7
DVE_PASSES_PER_IT = UNITS * 4
HACC_COLS = UNITS * 8           # 192


def _build_schedules(R):
    """Static per-engine instruction schedules with absolute indices."""
    act_prog, dve_prog = [], []
    for r in range(R):
        for i in range(N_UNITS):
            base = 24 * r + NQ * i
            for q in range(NQ):
                act_prog.append(("tau", base + q))
            for q in range(NQ):
                g = base + q
                act_prog += [("s0", g), ("s4", g), ("d3", g), ("d7", g)]
        for u in range(UNITS):
            g = 24 * r + u
            dve_prog += [("r1", g), ("r2", g), ("r5", g), ("r6", g)]
    act_index = {key: idx for idx, key in enumerate(act_prog)}
    dve_index = {key: idx for idx, key in enumerate(dve_prog)}
    return act_prog, dve_prog, act_index, dve_index


def _build_program(R=1):
    import concourse.bass as bass
    import concourse.mybir as mybir

    act_prog, dve_prog, act_index, dve_index = _build_schedules(R)

    nc = bass.Bass()
    xs = [
        nc.dram_tensor(f"x{u}", [128, FREE], mybir.dt.float32, kind="ExternalInput")
        for u in range(N_UNITS)
    ]
    cst = nc.dram_tensor("consts", [128, 16], mybir.dt.float32, kind="ExternalInput")
    hist_out = nc.dram_tensor(
        "hist", [128, 16], mybir.dt.float32, kind="ExternalOutput"
    )

    mult = mybir.AluOpType.mult

    from contextlib import ExitStack

    with ExitStack() as stack:
        def sb(name, shape):
            return stack.enter_context(nc.sbuf_tensor(name, shape, mybir.dt.float32))

        xts = [sb(f"xt{i}", [128, QF]) for i in range(6)]
        taus = [sb(f"tau{i}", [128, QF]) for i in range(4)]
        sas = [sb(f"sa{i}", [128, QF]) for i in range(2)]
        sbs = [sb(f"sb{i}", [128, QF]) for i in range(2)]
        p0s = [sb(f"p0{i}", [128, QF]) for i in range(2)]
        wscr = sb("wscr", [128, QF])
        cstt = sb("cstt", [128, 16])
        hacc = sb("hacc", [128, HACC_COLS])
        ho = sb("ho", [128, 16])
        ph0 = stack.enter_context(nc.psum_tensor("ph0", [128, 8], mybir.dt.float32))
        ph1 = stack.enter_context(nc.psum_tensor("ph1", [64, 8], mybir.dt.float32))
        sem_c = stack.enter_context(nc.semaphore("sem_c"))
        xsems = [stack.enter_context(nc.semaphore(f"sem_x{i}")) for i in range(6)]
        act_sem = stack.enter_context(nc.semaphore("act_sem"))
        dve_sem = stack.enter_context(nc.semaphore("dve_sem"))
        pe_sem = stack.enter_context(nc.semaphore("pe_sem"))
        cp_sem = stack.enter_context(nc.semaphore("cp_sem"))
        block = stack.enter_context(nc.Block())

        def img_q(g):
            return (g % 24) // NQ, g % NQ

        def col(g, rnd):
            return 8 * (g % 24) + rnd

        @block.sync
        def _(sync):
            sync.dma_start(out=cstt[:], in_=cst[:]).then_inc(sem_c, 16)
            for r in range(R):
                for u in range(UNITS):
                    g = 24 * r + u
                    slot = g % 6
                    i, q = img_q(g)
                    if g >= 6:
                        # xt slot free once unit g-6's last ACT read (d7) done
                        sync.wait_ge(act_sem, act_index[("d7", g - 6)] + 1)
                    sync.dma_start(
                        out=xts[slot][:], in_=xs[i][:, QF * q : QF * (q + 1)]
                    ).then_inc(xsems[slot], 16)
                sync.wait_ge(cp_sem, 2 * (r + 1))
                sync.dma_start(out=hist_out[:], in_=ho[:]).then_inc(sem_c, 16)

        @block.scalar
        def _(scalar):
            # dummy activation: pulls the exp table load forward
            scalar.activation(
                wscr[0:128, 0:1], wscr[0:128, 1:2],
                mybir.ActivationFunctionType.Exp,
                bias=0.0, scale=1.0,
            )
            scalar.wait_ge(sem_c, 16)
            for idx, (kind, g) in enumerate(act_prog):
                slot, q, b = g % 6, g % 4, g % 2
                r = g // 24
                # input quarter for this unit must be resident
                scalar.wait_ge(xsems[slot], 16 * (g // 6 + 1))
                if kind == "tau":
                    if g >= 4:
                        # tau slot free once unit g-4's last DVE read (r6) done
                        scalar.wait_ge(dve_sem, dve_index[("r6", g - 4)] + 1)
                    ins = scalar.activation(
                        taus[q][:], xts[slot][:],
                        mybir.ActivationFunctionType.Exp,
                        bias=0.0, scale=float(TAU_SCALE),
                    )
                elif kind == "s0":
                    if g % 24 == 0 and r > 0:
                        # hacc reused across iterations; PE must have read it
                        scalar.wait_ge(pe_sem, r)
                    if g >= 2:
                        # sa bank rewritten by DVE r2 of unit g-2
                        scalar.wait_ge(dve_sem, dve_index[("r2", g - 2)] + 1)
                    ins = scalar.activation(
                        sas[b][:], xts[slot][:],
                        mybir.ActivationFunctionType.Derivative_Erf,
                        bias=cstt[:, 0:1], scale=float(SCALE),
                        accum_out=hacc[:, col(g, 0) : col(g, 0) + 1],
                    )
                elif kind == "s4":
                    if g >= 2:
                        scalar.wait_ge(dve_sem, dve_index[("r6", g - 2)] + 1)
                    ins = scalar.activation(
                        sbs[b][:], xts[slot][:],
                        mybir.ActivationFunctionType.Derivative_Erf,
                        bias=cstt[:, 2:3], scale=float(SCALE),
                        accum_out=hacc[:, col(g, 4) : col(g, 4) + 1],
                    )
                elif kind == "d3":
                    ins = scalar.activation(
                        wscr[:], xts[slot][:],
                        mybir.ActivationFunctionType.Derivative_Erf,
                        bias=cstt[:, 1:2], scale=float(SCALE),
                        accum_out=hacc[:, col(g, 3) : col(g, 3) + 1],
                    )
                else:  # d7
                    ins = scalar.activation(
                        wscr[:], xts[slot][:],
                        mybir.ActivationFunctionType.Derivative_Erf,
                        bias=cstt[:, 3:4], scale=float(SCALE),
                        accum_out=hacc[:, col(g, 7) : col(g, 7) + 1],
                    )
                ins.then_inc(act_sem, 1)

        @block.vector
        def _(vector):
            for idx, (kind, g) in enumerate(dve_prog):
                q, b = g % 4, g % 2
                r = g // 24
                if kind == "r1":
                    vector.wait_ge(act_sem, act_index[("s0", g)] + 1)
                    ins = vector.scalar_tensor_tensor(
                        p0s[b][:], sas[b][:], cstt[:, 4:5], taus[q][:],
                        mult, mult,
                        accum_out=hacc[:, col(g, 1) : col(g, 1) + 1],
                    )
                elif kind == "r2":
                    ins = vector.scalar_tensor_tensor(
                        sas[b][:], p0s[b][:], cstt[:, 5:6], taus[q][:],
                        mult, mult,
                        accum_out=hacc[:, col(g, 2) : col(g, 2) + 1],
                    )
                elif kind == "r5":
                    vector.wait_ge(act_sem, act_index[("s4", g)] + 1)
                    ins = vector.scalar_tensor_tensor(
                        p0s[b][:], sbs[b][:], cstt[:, 6:7], taus[q][:],
                        mult, mult,
                        accum_out=hacc[:, col(g, 5) : col(g, 5) + 1],
                    )
                else:  # r6
                    ins = vector.scalar_tensor_tensor(
                        sbs[b][:], p0s[b][:], cstt[:, 7:8], taus[q][:],
                        mult, mult,
                        accum_out=hacc[:, col(g, 6) : col(g, 6) + 1],
                    )
                ins.then_inc(dve_sem, 1)
                if kind == "r6" and g % 24 == 23:
                    # end of iteration: copy PE results out (PE waits all accums)
                    it = g // 24
                    vector.wait_ge(pe_sem, it + 1)
                    vector.wait_ge(sem_c, 16 * (it + 1))
                    vector.tensor_copy(ho[:, 0:8], ph0[:, :]).then_inc(cp_sem, 1)
                    vector.tensor_copy(ho[0:64, 8:16], ph1[:, :]).then_inc(
                        cp_sem, 1
                    )

        @block.tensor
        def _(tensor):
            for r in range(R):
                tensor.wait_ge(act_sem, ACT_PASSES_PER_IT * (r + 1))
                tensor.wait_ge(dve_sem, DVE_PASSES_PER_IT * (r + 1))
                tensor.matmul(
                    ph0[0:128, 0:8], hacc[:, 0:128], cstt[:, 8:16],
                    start=True, stop=True,
                )
                tensor.matmul(
                    ph1[0:64, 0:8], hacc[:, 128:192], cstt[:, 8:16],
                    start=True, stop=True,
                ).then_inc(pe_sem, 1)

    return nc


def _make_consts():
    centers = np.linspace(0.0, 1.0, NUM_BINS).astype(np.float64)
    p = np.arange(128)
    k = p // 16
    cst = np.zeros((128, 16), dtype=np.float64)
    # Derivative_Erf bias columns for rounds 0, 3, 4, 7 (block k -> bin 8k+r)
    for ci, r in zip((0, 1, 2, 3), (0, 3, 4, 7)):
        cst[:, ci] = -centers[8 * k + r] * SCALE
    # recurrence constants C_{j-1->j} for rounds 1, 2, 5, 6
    for ci, r in zip((4, 5, 6, 7), (1, 2, 5, 6)):
        j = 8 * k + r
        cst[:, ci] = np.exp(-SPACING * (centers[j - 1] + centers[j]) / DENOM)
    # block selector for the PE reduce
    for kk in range(8):
        cst[k == kk, 8 + kk] = 1.0
    return cst.astype(np.float32)


@functools.lru_cache(maxsize=1)
def _get_runner():
    """Compile the SPMD program once; return a callable list[in_map] -> list[out_map]."""
    import jax
    from jax.experimental.shard_map import shard_map
    from jax.sharding import Mesh, PartitionSpec

    from concourse import mybir
    from concourse.bass2jax import (
        _bass_exec_p,
        install_neuronx_cc_hook,
        partition_id_tensor,
    )

    nc = _build_program()
    install_neuronx_cc_hook()

    partition_name = (
        nc.partition_id_tensor.name if nc.partition_id_tensor else None
    )
    in_names, out_names, out_avals, zero_outs = [], [], [], []
    for alloc in nc.m.functions[0].allocations:
        if not isinstance(alloc, mybir.MemoryLocationSet):
            continue
        name = alloc.memorylocations[0].name
        if alloc.kind == "ExternalInput":
            if name != partition_name:
                in_names.append(name)
        elif alloc.kind == "ExternalOutput":
            out_names.append(name)
            shape = tuple(alloc.tensor_shape)
            dtype = mybir.dt.np(alloc.dtype)
            out_avals.append(jax.core.ShapedArray(shape, dtype))
            zero_outs.append(np.zeros(shape, dtype))
    n_params = len(in_names)
    n_outs = len(out_avals)
    all_in_names = list(in_names) + list(out_names)
    if partition_name is not None:
        all_in_names.append(partition_name)
    donate = tuple(range(n_params, n_params + n_outs))

    def _body(*args):
        operands = list(args)
        if partition_name is not None:
            operands.append(partition_id_tensor())
        outs = _bass_exec_p.bind(
            *operands,
            out_avals=tuple(out_avals),
            in_names=tuple(all_in_names),
            out_names=tuple(out_names),
            lowering_input_output_aliases=(),
            sim_require_finite=True,
            sim_require_nnan=True,
            nc=nc,
        )
        return tuple(outs)

    devices = jax.devices()[:N_CORES]
    mesh = Mesh(np.asarray(devices), ("core",))
    sharded = jax.jit(
        shard_map(
            _body,
            mesh=mesh,
            in_specs=(PartitionSpec("core"),) * (n_params + n_outs),
            out_specs=(PartitionSpec("core"),) * n_outs,
            check_rep=False,
        ),
        donate_argnums=donate,
        keep_unused=True,
    )

    class Runner:
        def __init__(self):
            self.sharded = sharded
            self.in_names = in_names
            self.out_names = out_names
            self.out_avals = out_avals
            self.zero_outs = zero_outs

        def concat_inputs(self, in_maps):
            return [
                np.concatenate([np.asarray(m[name]) for m in in_maps], axis=0)
                for name in in_names
            ]

        def fresh_zeros(self):
            return [
                np.zeros((N_CORES * z.shape[0], *z.shape[1:]), z.dtype)
                for z in zero_outs
            ]

        def split_outputs(self, out_arrs):
            return [
                {
                    name: np.asarray(out_arrs[i]).reshape(
                        N_CORES, *out_avals[i].shape
                    )[c]
                    for i, name in enumerate(out_names)
                }
                for c in range(N_CORES)
            ]

        def __call__(self, in_maps):
            out_arrs = self.sharded(*self.concat_inputs(in_maps), *self.fresh_zeros())
            return self.split_outputs(out_arrs)

    return Runner()


def _shard_inputs(pred, target):
    cst = _make_consts()
    maps = []
    for b in range(B):
        m = {"consts": cst}
        for c in range(C):
            for t, src in enumerate((pred, target)):
                u = 2 * c + t
                img = np.ascontiguousarray(src[b, c], dtype=np.float32).reshape(
                    16, FREE
                )
                m[f"x{u}"] = np.tile(img, (8, 1))
        maps.append(m)
    return maps


def _unpack_hist(ho):
    """ho [128, 16] -> hist [N_UNITS, NUM_BINS] (float64)."""
    ho = ho.astype(np.float64)
    hist = np.zeros((N_UNITS, NUM_BINS), dtype=np.float64)
    for u in range(UNITS):
        i, q = u // NQ, u % NQ
        for rnd in range(8):
            c = 8 * u + rnd
            vals = ho[c, 0:8] if c < 128 else ho[c - 128, 8:16]
            for k in range(8):
                hist[i, 8 * k + rnd] += vals[k]
    return hist


def _finish_on_host(results):
    total = 0.0
    for b in range(B):
        hist = _unpack_hist(results[b]["hist"]) * DERF_SCALE
        for c in range(C):
            pcs = hist[2 * c]
            tcs = hist[2 * c + 1]
            pn = pcs / (pcs.sum() + 1e-7)
            tn = tcs / (tcs.sum() + 1e-7)
            total += np.abs(np.cumsum(pn) - np.cumsum(tn)).sum()
    return np.float32(total / (B * C * NUM_BINS))


def kernel(pred, target):
    pred = np.asarray(pred, dtype=np.float32)
    target = np.asarray(target, dtype=np.float32)
    assert pred.shape == (B, C, H, W) and target.shape == (B, C, H, W)
    run = _get_runner()
    results = run(_shard_inputs(pred, target))
    return np.asarray(_finish_on_host(results), dtype=np.float32)
